# revision 13
# baseline (speedup 1.0000x reference)
"""Trainium2 Bass kernel for nn_BinConv2d: BN(train-mode) -> sign -> 3x3 conv.

Strategy (8 NeuronCores):
  Launch A (stats): channel-sharded. Core c gets x[:, 8c:8c+8] viewed as
    [128 partitions = 16 img x 8 ch, 50176]. bn_stats/bn_aggr produce per
    (img, ch) mean/var.  Host pools them (f64) into per-channel batch
    mean/var, then folds BN+sign into a per-channel threshold:
        sign(BN(x)) = sign(x - t_c),  t_c = mean_c - bias_c*sqrt(var_c+eps)/w_c
    (valid since bn_weight > 0 and rsqrt > 0).
  Launch B (conv): batch-sharded. Core c gets images [2c, 2c+1] as
    [128 partitions = 2 img x 64 ch, 224, 224].  Per 28-row band:
    DMA in -> ACT Sign(x - t) into a width-padded fp16 "strip"
    [128, 30 rows x 226 cols] -> 3x3 conv as 9 shifted matmuls accumulated
    in PSUM (tap (dy,dx) reads the strip at offset (2i+dy)*226+dx-1).
    Image 0 runs on PE quadrant (rows 0:64, cols 0:64), image 1 on
    (64:128, 64:128) - the two 64x64 matmuls execute concurrently in
    disjoint sub-arrays.  DVE evacuates PSUM (+conv bias) to a staging
    tile, DMA out.
"""

import sys

if "/opt/trn_rl_repo" not in sys.path:
    sys.path.insert(0, "/opt/trn_rl_repo")

import numpy as np

import concourse.bacc as bacc
import concourse.tile as tile
from concourse import mybir
from concourse.bass_utils import run_bass_kernel_spmd

F32 = mybir.dt.float32
F16 = mybir.dt.float16

N_CORES = 8
N, C, H, W = 16, 64, 224, 224
BN_EPS = 1e-4
BAND = 28              # output rows per band
NB = H // BAND         # 8 bands
WP = W + 2             # padded strip width (226)
NT = BAND // 2         # 14 tiles per band, 2 output rows (452 cols) each
MM_N = 2 * WP          # 452 matmul free dim
STRIP_LEN = (BAND + 2) * WP  # 30 rows x 226


def build_stats_nc(repeat=1):
    """Per-core: x_s [128, 50176] f32 -> stats [128, 2] (mean, var)."""
    nc = bacc.Bacc()
    cols = H * W
    x_s = nc.declare_dram_parameter("x_s", [128, cols], F32, isOutput=False)
    stats_out = nc.declare_dram_parameter("stats", [128, 2], F32, isOutput=True)

    n_groups = cols // 512  # 98
    # split groups into 8 chunks for DMA pipelining
    n_chunks = 8
    base = n_groups // n_chunks
    rem = n_groups - base * n_chunks
    chunk_groups = [base + (1 if i >= n_chunks - rem else 0) for i in range(n_chunks)]

    with tile.TileContext(nc) as tc:
        with (
            tc.tile_pool(name="xc", bufs=3) as xpool,
            tc.tile_pool(name="acc", bufs=1) as apool,
        ):
            stats = apool.tile([128, n_groups, 6], F32)
            mv = apool.tile([128, 2], F32)

            def emit_all():
                g0 = 0
                for ci in range(n_chunks):
                    ng = chunk_groups[ci]
                    xt = xpool.tile([128, (base + 1) * 512], F32, tag="xt")
                    nc.sync.dma_start(
                        out=xt[:, : ng * 512],
                        in_=x_s[:, g0 * 512 : (g0 + ng) * 512],
                    )
                    for g in range(ng):
                        nc.vector.bn_stats(
                            out=stats[:, g0 + g, :],
                            in_=xt[:, g * 512 : (g + 1) * 512],
                        )
                    g0 += ng
                nc.vector.bn_aggr(out=mv[:], in_=stats[:])
                nc.sync.dma_start(out=stats_out[:], in_=mv[:])

            if repeat == 1:
                emit_all()
            else:
                with tc.For_i(0, repeat, 1):
                    emit_all()
    nc.compile()
    return nc


def build_conv_nc(repeat=1):
    """Per-core conv kernel: x_b [128, 224, 224] f32 (2 img x 64 ch),
    wts [128, 9, 64] fp16, tneg [128,1] f32, cbias [128,1] f32
    -> y [128, 224, 224] f32."""
    nc = bacc.Bacc()
    x_b = nc.declare_dram_parameter("x_b", [128, H, W], F32, isOutput=False)
    wts = nc.declare_dram_parameter("wts", [128, 9, C], F16, isOutput=False)
    tneg = nc.declare_dram_parameter("tneg", [128, 1], F32, isOutput=False)
    cbias = nc.declare_dram_parameter("cbias", [128, 1], F32, isOutput=False)
    y = nc.declare_dram_parameter("y", [128, H, W], F32, isOutput=True)

    with tile.TileContext(nc) as tc:
        with (
            tc.tile_pool(name="const", bufs=1) as cpool,
            tc.tile_pool(name="xband", bufs=3) as xpool,
            tc.tile_pool(name="strips", bufs=1) as spool,
            tc.tile_pool(name="stage", bufs=2) as opool,
            tc.tile_pool(name="psum", bufs=8, space="PSUM") as ppool,
        ):
            wsb = cpool.tile([128, 9, C], F16)
            nc.sync.dma_start(out=wsb[:], in_=wts[:])
            tsb = cpool.tile([128, 1], F32)
            nc.sync.dma_start(out=tsb[:], in_=tneg[:])
            bsb = cpool.tile([128, 1], F32)
            nc.sync.dma_start(out=bsb[:], in_=cbias[:])

            # two persistent strip buffers (manual double buffer), 1 fp16 pad
            # element on each end so edge taps of edge tiles stay in-bounds.
            strips = [
                cpool.tile([128, STRIP_LEN + 2], F16, name=f"strip{i}",
                           tag=f"strip{i}")
                for i in range(2)
            ]
            for s in strips:
                nc.vector.memset(s[:], 0.0)

            def load_band(b, xts):
                r0 = b * BAND
                lo = max(r0 - 1, 0)
                hi = min(r0 + BAND + 1, H)
                s0 = lo - (r0 - 1)  # strip row slot of first loaded row
                xt = xpool.tile([128, BAND + 2, W], F32, name=f"xt{b % 3}",
                                tag="xt")
                nc.sync.dma_start(out=xt[:, s0 : s0 + (hi - lo), :],
                                  in_=x_b[:, lo:hi, :])
                xts[b] = (xt, s0, hi - lo)

            def emit_all():
                xts = {}
                for b in range(NB):
                    load_band(b, xts)
                    r0 = b * BAND
                    strip = strips[b % 2]
                    sflat = strip[:, 1 : 1 + STRIP_LEN]
                    s3 = sflat.rearrange("p (a b) -> p a b", b=WP)
                    xt, s0, nr = xts.pop(b)

                    if b == NB - 1:
                        # bottom pad row (buffer previously held real data)
                        nc.vector.memset(s3[:, BAND + 1, :], 0.0)
                    nc.scalar.activation(
                        out=s3[:, s0 : s0 + nr, 1 : 1 + W],
                        in_=xt[:, s0 : s0 + nr, :],
                        func=mybir.ActivationFunctionType.Sign,
                        bias=tsb[:],
                    )

                    stg = opool.tile([128, BAND, W], F32, tag="stg")
                    # groups of 4 tiles: 2 groups of psum banks in flight, so
                    # the PE never waits a whole half-band for DVE evacuation
                    for g0 in range(0, NT, 4):
                        tiles = range(g0, min(g0 + 4, NT))
                        psums = {
                            i: ppool.tile([128, 512], F32, name=f"ps{i}",
                                          tag="ps")
                            for i in tiles
                        }
                        # tap-outer, quadrant-blocked: consecutive matmuls
                        # share weights; alternating quadrant blocks let the
                        # weight load of one quadrant hide under the other's
                        # streaming (measured ~106 ns/MM vs 420 serialized)
                        for t in range(9):
                            dy, dx = t // 3, t % 3
                            for q in (0, 64):
                                for i in tiles:
                                    st = (2 * i + dy) * WP + dx  # +1 pad -1 tap
                                    rhs = strip[:, st : st + MM_N]
                                    nc.tensor.matmul(
                                        psums[i][q : q + 64, :MM_N],
                                        wsb[q : q + 64, t, :],
                                        rhs[q : q + 64, :],
                                        start=(t == 0),
                                        stop=(t == 8),
                                        # sim's zero-region tracker can't see
                                        # that the two quadrants write
                                        # disjoint partition ranges
                                        skip_group_check=True,
                                    )
                        for i in tiles:
                            ps3 = psums[i][:, :MM_N].rearrange(
                                "p (r c) -> p r c", c=WP
                            )
                            nc.vector.tensor_scalar(
                                out=stg[:, 2 * i : 2 * i + 2, :],
                                in0=ps3[:, :, 1 : 1 + W],
                                scalar1=bsb[:],
                                scalar2=None,
                                op0=mybir.AluOpType.add,
                            )
                    nc.sync.dma_start(out=y[:, r0 : r0 + BAND, :], in_=stg[:])

            if repeat == 1:
                emit_all()
            else:
                with tc.For_i(0, repeat, 1):
                    emit_all()
    nc.compile()
    return nc


_cache = {}


def _get(name, builder):
    if name not in _cache:
        _cache[name] = builder()
    return _cache[name]


def _prep_conv_inputs(x, bn_weight, bn_bias, conv_weight, conv_bias, stats):
    # pool per-(img,ch) stats -> per-channel batch stats (f64)
    ipc = N // N_CORES
    meanM = np.empty((N, C), np.float64)
    varM = np.empty((N, C), np.float64)
    for c in range(N_CORES):
        s = stats[c].astype(np.float64).reshape(ipc, C, 2)
        meanM[ipc * c : ipc * (c + 1)] = s[..., 0]
        varM[ipc * c : ipc * (c + 1)] = s[..., 1]
    m = meanM.mean(axis=0)
    v = (varM + meanM**2).mean(axis=0) - m**2
    t = m - bn_bias.astype(np.float64) * np.sqrt(v + BN_EPS) / bn_weight.astype(
        np.float64
    )
    tneg = np.tile((-t).astype(np.float32), 2)[:, None]  # [128,1]
    cb = np.tile(conv_bias.astype(np.float32), 2)[:, None]
    # lhsT[t][cin, cout] = conv_weight[cout, cin, dy, dx]; dup on both halves
    w9 = conv_weight.transpose(1, 2, 3, 0).reshape(C, 9, C)  # [cin, tap, cout]
    wts = np.concatenate([w9, w9], axis=0).astype(np.float16)
    return wts, tneg, cb


def kernel(x, bn_weight, bn_bias, conv_weight, conv_bias):
    x = np.ascontiguousarray(x, dtype=np.float32)

    ipc = N // N_CORES
    nc_s = _get("stats", build_stats_nc)
    in_maps = [
        {"x_s": x[ipc * c : ipc * (c + 1)].reshape(128, H * W)}
        for c in range(N_CORES)
    ]
    res = run_bass_kernel_spmd(nc_s, in_maps, list(range(N_CORES))).results
    stats = [res[c]["stats"] for c in range(N_CORES)]

    wts, tneg, cb = _prep_conv_inputs(
        x, bn_weight, bn_bias, conv_weight, conv_bias, stats
    )

    nc_c = _get("conv", build_conv_nc)
    in_maps = [
        {
            "x_b": x[ipc * c : ipc * (c + 1)].reshape(128, H, W),
            "wts": wts,
            "tneg": tneg,
            "cbias": cb,
        }
        for c in range(N_CORES)
    ]
    res = run_bass_kernel_spmd(nc_c, in_maps, list(range(N_CORES))).results
    y = np.concatenate(
        [res[c]["y"].reshape(ipc, C, H, W) for c in range(N_CORES)], axis=0
    )
    return y
